# revision 1
# baseline (speedup 1.0000x reference)
"""Binary 3-layer CNN (sign activations + sign weights) on 8 NeuronCores.

Strategy: pure data parallel — 64 images -> 8 cores x 8 images.
Per core: 2 batches of 4 images; SBUF partition layout [128 = (4 img, 32 ch), pix].
Each 3x3 conv = 9 PSUM-accumulating matmuls with block-diagonal stationary
weights (4 identical 32x32 blocks) and free-dim-shifted rhs APs (dy*Wpad+dx),
so there is no im2col data movement. All matmul operands are exactly +-1/0 ->
bf16 with fp32 PSUM accumulation is numerically exact. sign() is applied by
ScalarE during PSUM->SBUF eviction. Layers staged through DRAM scratch in a
zero-padded layout (pad rows AND pad cols pre-zeroed in DRAM once) so conv
padding is baked in and SBUF tiles are single-producer.
"""

import numpy as np
import ml_dtypes

import concourse.bass as bass
import concourse.mybir as mybir
import concourse.tile as tile
from concourse import bacc
from concourse.bass_utils import run_bass_kernel_spmd

BF16 = mybir.dt.bfloat16
F32 = mybir.dt.float32
AF = mybir.ActivationFunctionType

N_CORES = 8
IMG_PER_CORE = 8
B = 4          # images per partition-batch
H = W = 256
WP = 258       # padded width (1 col pad each side)
HP = 258       # padded height
R = 64         # strip rows (stages A/B)
RC = 32        # strip rows (stage C)
NB = IMG_PER_CORE // B  # batches per core


def _conv_strip(nc, pspool, lhsT_taps, hin, dst_evict, rows):
    """rows output rows; hin is [*, rows+2, WP] (padded); evict 4 psum tiles."""
    mparts = lhsT_taps[0].shape[-1]
    for r0 in range(0, rows, 8):  # groups of 4 psum tiles (2 rows each)
        pss = [pspool.tile([mparts, 2, 256], F32, tag=f"ps{p}", name=f"ps{p}")
               for p in range(4)]
        for tap in range(9):
            dy, dx = tap // 3, tap % 3
            for p in range(4):
                r = r0 + 2 * p
                nc.tensor.matmul(
                    pss[p][:, :, :],
                    lhsT_taps[tap],
                    hin[:, r + dy:r + dy + 2, dx:dx + 256],
                    start=(tap == 0), stop=(tap == 8))
        dst_evict(pss, r0)


def _build_program(stages=('0','A','B','C')):
    nc = bacc.Bacc("TRN2", target_bir_lowering=False, debug=False)

    x_in = nc.dram_tensor("x", [IMG_PER_CORE, H, W], F32, kind="ExternalInput")
    s0_in = nc.dram_tensor("s0", [12, 3, 128], BF16, kind="ExternalInput")
    s1_in = nc.dram_tensor("s1", [128, 9, 128], BF16, kind="ExternalInput")
    s2_in = nc.dram_tensor("s2", [128, 9, B], BF16, kind="ExternalInput")
    out_d = nc.dram_tensor("out", [IMG_PER_CORE, H, W], F32, kind="ExternalOutput")

    xs_d = nc.dram_tensor("xs", [IMG_PER_CORE, HP, WP], BF16)
    h0_d = nc.dram_tensor("h0", [NB, 128, HP, WP], BF16)
    h1_d = nc.dram_tensor("h1", [NB, 128, HP, WP], BF16)

    with tile.TileContext(nc) as tc:
        with (
            tc.tile_pool(name="const", bufs=1) as cpool,
            tc.tile_pool(name="xprep", bufs=4) as xpool,
            tc.tile_pool(name="ain", bufs=2) as apool,
            tc.tile_pool(name="aout", bufs=2) as opool,
            tc.tile_pool(name="cout", bufs=1) as cpool2,
            tc.tile_pool(name="psum", bufs=2, space="PSUM") as pspool,
        ):
            # --- constants: stationary weights + a zero tile ---
            # s0 replicated into 4 row groups (base partitions 0/32/64/96)
            # so 4 psum tiles' conv0 matmuls run concurrently via row tiling
            s0t = cpool.tile([108, 3, 128], BF16, tag="s0")
            for p in range(4):
                nc.sync.dma_start(out=s0t[32 * p:32 * p + 12, :, :],
                                  in_=s0_in[:, :, :])
            s1t = cpool.tile([128, 9, 128], BF16, tag="s1")
            nc.sync.dma_start(out=s1t[:, :, :], in_=s1_in[:, :, :])
            s2t = cpool.tile([128, 9, B], BF16, tag="s2")
            nc.sync.dma_start(out=s2t[:, :, :], in_=s2_in[:, :, :])
            zt = cpool.tile([128, WP], BF16, tag="zt")
            nc.gpsimd.memset(zt[:, :], 0.0)

            # --- pre-zero DRAM pad rows (contiguous); col pads are baked
            # into the SBUF tiles below ---
            for img in range(IMG_PER_CORE):
                nc.scalar.dma_start(out=xs_d[img, 0:1, :], in_=zt[0:1, :])
                nc.scalar.dma_start(out=xs_d[img, HP - 1:HP, :], in_=zt[0:1, :])
            for b in range(NB):
                for hd in (h0_d, h1_d):
                    nc.scalar.dma_start(out=hd[b, :, 0, :], in_=zt[:, :])
                    nc.scalar.dma_start(out=hd[b, :, HP - 1, :], in_=zt[:, :])

            # --- stage 0: sign(x) -> padded bf16 planes in DRAM ---
            for img in range(IMG_PER_CORE if '0' in stages else 0):
                for rb in range(H // 128):
                    xf = xpool.tile([128, W], F32, tag="xf")
                    nc.sync.dma_start(
                        out=xf[:, :], in_=x_in[img, rb * 128:(rb + 1) * 128, :])
                    xp = xpool.tile([128, WP], BF16, tag="xp")
                    nc.scalar.activation(xp[:, 1:W + 1], xf[:, :], AF.Sign)
                    nc.vector.memset(xp[:, 0:1], 0.0)
                    nc.vector.memset(xp[:, WP - 1:WP], 0.0)
                    nc.scalar.dma_start(
                        out=xs_d[img, rb * 128 + 1:(rb + 1) * 128 + 1, :],
                        in_=xp[:, :])

            for b in range(NB):
                # ---- stage A: conv0 (1 -> 32ch), dy-in-K: K=12, M=128,
                # 4x row tiling: input replicated to partition groups
                # 0/32/64/96; the 4 psum tiles' matmuls occupy distinct
                # 32-row strips of the PE array and run concurrently ----
                for s in range(H // R if 'A' in stages else 0):
                    xt = apool.tile([108, R, WP], BF16, tag="lin")
                    for p in range(4):
                        for dy in range(3):
                            nc.sync.dma_start(
                                out=xt[32 * p + dy * B:32 * p + (dy + 1) * B,
                                       :, :],
                                in_=xs_d[b * B:(b + 1) * B,
                                         s * R + dy:s * R + dy + R, :])
                    ht = opool.tile([128, R, WP], BF16, tag="a_out")
                    nc.vector.memset(ht[:, :, 0:1], 0.0)
                    nc.vector.memset(ht[:, :, WP - 1:WP], 0.0)
                    for r0 in range(0, R, 8):
                        pss = [pspool.tile([128, 2, 256], F32,
                                           tag=f"ps{p}", name=f"ps{p}")
                               for p in range(4)]
                        for dx in range(3):
                            for p in range(4):
                                r = r0 + 2 * p
                                nc.tensor.matmul(
                                    pss[p][:, :, :],
                                    s0t[32 * p:32 * p + 12, dx, :],
                                    xt[32 * p:32 * p + 12, r:r + 2,
                                       dx:dx + 256],
                                    start=(dx == 0), stop=(dx == 2),
                                    tile_position=(32 * p, 0))
                        for p in range(4):
                            r = r0 + 2 * p
                            nc.scalar.activation(
                                ht[:, r:r + 2, 1:W + 1], pss[p][:, :, :], AF.Sign)
                    nc.scalar.dma_start(
                        out=h0_d[b, :, s * R + 1:s * R + R + 1, :],
                        in_=ht[:, :, :])

                # ---- stage B: conv1 (32 -> 32ch), K=128, M=128 ----
                for s in range(H // R if 'B' in stages else 0):
                    hin = apool.tile([128, R + 2, WP], BF16, tag="lin")
                    nc.sync.dma_start(
                        out=hin[:, :, :], in_=h0_d[b, :, s * R:s * R + R + 2, :])
                    ht = opool.tile([128, R, WP], BF16, tag="a_out")
                    nc.vector.memset(ht[:, :, 0:1], 0.0)
                    nc.vector.memset(ht[:, :, WP - 1:WP], 0.0)

                    def evict_b(pss, r0, ht=ht):
                        for p in range(4):
                            r = r0 + 2 * p
                            nc.scalar.activation(
                                ht[:, r:r + 2, 1:W + 1], pss[p][:, :, :], AF.Sign)

                    _conv_strip(nc, pspool,
                                [s1t[:, t, :] for t in range(9)], hin, evict_b, R)
                    nc.scalar.dma_start(
                        out=h1_d[b, :, s * R + 1:s * R + R + 1, :],
                        in_=ht[:, :, :])

                # ---- stage C: conv2 (32 -> 1ch), K=128, M=4, 4x col-tiling ----
                # 4 psum row-pairs go to col groups 0/32/64/96 of the SAME
                # psum tile; the 4 matmuls per tap run concurrently on
                # distinct 32-col strips of the PE array.
                for s in range(H // RC if 'C' in stages else 0):
                    hin = apool.tile([128, RC + 2, WP], BF16, tag="lin")
                    nc.sync.dma_start(
                        out=hin[:, :, :], in_=h1_d[b, :, s * RC:s * RC + RC + 2, :])
                    ot = cpool2.tile([B, RC, W], F32, tag="c_out")
                    for r0 in range(0, RC, 8):
                        ps = pspool.tile([128, 2, 256], F32, tag="ps0", name="psc")
                        for tap in range(9):
                            dy, dx = tap // 3, tap % 3
                            for p in range(4):
                                r = r0 + 2 * p
                                nc.tensor.matmul(
                                    ps[32 * p:32 * p + B, :, :],
                                    s2t[:, tap, :],
                                    hin[:, r + dy:r + dy + 2, dx:dx + 256],
                                    start=(tap == 0), stop=(tap == 8),
                                    tile_position=(0, 32 * p))
                        for p in range(4):
                            r = r0 + 2 * p
                            nc.vector.tensor_copy(
                                ot[:, r:r + 2, :], ps[32 * p:32 * p + B, :, :])
                    for g in range(B):
                        nc.scalar.dma_start(
                            out=out_d[b * B + g, s * RC:s * RC + RC, :],
                            in_=ot[g:g + 1, :, :])
    nc.compile()
    return nc


def _host_weights(w0, w1, w2):
    """Build bf16 block-diag stationary matrices. tap index = dy*3+dx."""
    sg = lambda w: np.sign(w).astype(ml_dtypes.bfloat16)
    w0s, w1s, w2s = sg(w0), sg(w1), sg(w2)   # [32,1,3,3],[32,32,3,3],[1,32,3,3]
    s0 = np.zeros((12, 3, 128), ml_dtypes.bfloat16)
    s1 = np.zeros((128, 9, 128), ml_dtypes.bfloat16)
    s2 = np.zeros((128, 9, B), ml_dtypes.bfloat16)
    for g in range(B):
        for dy in range(3):
            for dx in range(3):
                t = dy * 3 + dx
                # lhsT[k, m]: out[m] += sum_k lhsT[k,m]*rhs[k]
                # s0 [dy*4+g, dx, g*32+co]
                s0[dy * B + g, dx, g * 32:(g + 1) * 32] = w0s[:, 0, dy, dx]
                s1[g * 32:(g + 1) * 32, t, g * 32:(g + 1) * 32] = \
                    w1s[:, :, dy, dx].T  # [ci, co]
                s2[g * 32:(g + 1) * 32, t, g] = w2s[0, :, dy, dx]
    return s0, s1, s2


_NC_CACHE = {}


def kernel(x, w0, w1, w2):
    if "nc" not in _NC_CACHE:
        _NC_CACHE["nc"] = _build_program()
    nc = _NC_CACHE["nc"]
    s0, s1, s2 = _host_weights(np.asarray(w0), np.asarray(w1), np.asarray(w2))
    x = np.asarray(x, np.float32).reshape(64, H, W)
    in_maps = [
        {"x": np.ascontiguousarray(x[i * IMG_PER_CORE:(i + 1) * IMG_PER_CORE]),
         "s0": s0, "s1": s1, "s2": s2}
        for i in range(N_CORES)
    ]
    res = run_bass_kernel_spmd(nc, in_maps, list(range(N_CORES)))
    out = np.stack([np.asarray(res.results[i]["out"], np.float32)
                    for i in range(N_CORES)])
    return out.reshape(64, 1, H, W)



# revision 2
# speedup vs baseline: 11.1197x; 11.1197x over previous
"""Binary 3-layer CNN (sign activations + sign weights) on 8 NeuronCores.

Strategy: pure data parallel — 64 images -> 8 cores x 8 images.
Per core: 2 batches of 4 images; SBUF partition layout [128 = (4 img, 32 ch), pix].
Each 3x3 conv = 9 PSUM-accumulating matmuls with block-diagonal stationary
weights (4 identical 32x32 blocks) and free-dim-shifted rhs APs (dy*Wpad+dx),
so there is no im2col data movement. All matmul operands are exactly +-1/0 ->
bf16 with fp32 PSUM accumulation is numerically exact. sign() is applied by
ScalarE during PSUM->SBUF eviction. Layers staged through DRAM scratch in a
zero-padded layout (pad rows AND pad cols pre-zeroed in DRAM once) so conv
padding is baked in and SBUF tiles are single-producer.
"""

import numpy as np
import ml_dtypes

import concourse.bass as bass
import concourse.mybir as mybir
import concourse.tile as tile
from concourse import bacc
from concourse.bass_utils import run_bass_kernel_spmd

BF16 = mybir.dt.bfloat16
F32 = mybir.dt.float32
AF = mybir.ActivationFunctionType

N_CORES = 8
IMG_PER_CORE = 8
B = 4          # images per partition-batch
H = W = 256
WP = 258       # padded width (1 col pad each side)
HP = 258       # padded height
R = 64         # strip rows (stages A/B)
RC = 32        # strip rows (stage C)
NB = IMG_PER_CORE // B  # batches per core


def _conv_strip(nc, pspool, lhsT_taps, hin, dst_evict, rows):
    """rows output rows; hin is [*, rows+2, WP] (padded); evict 4 psum tiles."""
    mparts = lhsT_taps[0].shape[-1]
    for r0 in range(0, rows, 8):  # groups of 4 psum tiles (2 rows each)
        pss = [pspool.tile([mparts, 2, 256], F32, tag=f"ps{p}", name=f"ps{p}")
               for p in range(4)]
        for tap in range(9):
            dy, dx = tap // 3, tap % 3
            for p in range(4):
                r = r0 + 2 * p
                nc.tensor.matmul(
                    pss[p][:, :, :],
                    lhsT_taps[tap],
                    hin[:, r + dy:r + dy + 2, dx:dx + 256],
                    start=(tap == 0), stop=(tap == 8))
        dst_evict(pss, r0)


def _build_program(stages=('0','A','B','C')):
    nc = bacc.Bacc("TRN2", target_bir_lowering=False, debug=False)

    x_in = nc.dram_tensor("x", [IMG_PER_CORE, H, W], F32, kind="ExternalInput")
    s0_in = nc.dram_tensor("s0", [12, 3, 128], BF16, kind="ExternalInput")
    s1_in = nc.dram_tensor("s1", [128, 9, 128], BF16, kind="ExternalInput")
    s2_in = nc.dram_tensor("s2", [128, 9, B], BF16, kind="ExternalInput")
    out_d = nc.dram_tensor("out", [IMG_PER_CORE, H, W], F32, kind="ExternalOutput")

    xs_d = nc.dram_tensor("xs", [IMG_PER_CORE, HP, WP], BF16)
    h0_d = nc.dram_tensor("h0", [NB, 128, HP, WP], BF16)
    h1_d = nc.dram_tensor("h1", [NB, 128, HP, WP], BF16)

    with tile.TileContext(nc) as tc:
        with (
            tc.tile_pool(name="const", bufs=1) as cpool,
            tc.tile_pool(name="xprep", bufs=4) as xpool,
            tc.tile_pool(name="ain", bufs=2) as apool,
            tc.tile_pool(name="aout", bufs=2) as opool,
            tc.tile_pool(name="cout", bufs=1) as cpool2,
            tc.tile_pool(name="psum", bufs=2, space="PSUM") as pspool,
        ):
            # --- constants: stationary weights + a zero tile ---
            # s0 replicated into 4 row groups (base partitions 0/32/64/96)
            # so 4 psum tiles' conv0 matmuls run concurrently via row tiling
            s0t = cpool.tile([108, 3, 128], BF16, tag="s0")
            for p in range(4):
                nc.sync.dma_start(out=s0t[32 * p:32 * p + 12, :, :],
                                  in_=s0_in[:, :, :])
            s1t = cpool.tile([128, 9, 128], BF16, tag="s1")
            nc.sync.dma_start(out=s1t[:, :, :], in_=s1_in[:, :, :])
            s2t = cpool.tile([128, 9, B], BF16, tag="s2")
            nc.sync.dma_start(out=s2t[:, :, :], in_=s2_in[:, :, :])
            zt = cpool.tile([128, WP], BF16, tag="zt")
            nc.gpsimd.memset(zt[:, :], 0.0)

            # --- pre-zero DRAM pad rows (contiguous); col pads are baked
            # into the SBUF tiles below ---
            for img in range(IMG_PER_CORE):
                nc.scalar.dma_start(out=xs_d[img, 0:1, :], in_=zt[0:1, :])
                nc.scalar.dma_start(out=xs_d[img, HP - 1:HP, :], in_=zt[0:1, :])
            for b in range(NB):
                for hd in (h0_d, h1_d):
                    nc.scalar.dma_start(out=hd[b, :, 0, :], in_=zt[:, :])
                    nc.scalar.dma_start(out=hd[b, :, HP - 1, :], in_=zt[:, :])

            # --- stage 0: sign(x) -> padded bf16 planes in DRAM ---
            for img in range(IMG_PER_CORE if '0' in stages else 0):
                for rb in range(H // 128):
                    xf = xpool.tile([128, W], F32, tag="xf")
                    nc.sync.dma_start(
                        out=xf[:, :], in_=x_in[img, rb * 128:(rb + 1) * 128, :])
                    xp = xpool.tile([128, WP], BF16, tag="xp")
                    nc.scalar.activation(xp[:, 1:W + 1], xf[:, :], AF.Sign)
                    nc.vector.memset(xp[:, 0:1], 0.0)
                    nc.vector.memset(xp[:, WP - 1:WP], 0.0)
                    nc.scalar.dma_start(
                        out=xs_d[img, rb * 128 + 1:(rb + 1) * 128 + 1, :],
                        in_=xp[:, :])

            for b in range(NB):
                # ---- stage A: conv0 (1 -> 32ch), dy-in-K: K=12, M=128,
                # 4x row tiling: input replicated to partition groups
                # 0/32/64/96; the 4 psum tiles' matmuls occupy distinct
                # 32-row strips of the PE array and run concurrently ----
                for s in range(H // R if 'A' in stages else 0):
                    xt = apool.tile([108, R, WP], BF16, tag="lin")
                    for p in range(4):
                        for dy in range(3):
                            nc.sync.dma_start(
                                out=xt[32 * p + dy * B:32 * p + (dy + 1) * B,
                                       :, :],
                                in_=xs_d[b * B:(b + 1) * B,
                                         s * R + dy:s * R + dy + R, :])
                    ht = opool.tile([128, R, WP], BF16, tag="a_out")
                    nc.vector.memset(ht[:, :, 0:1], 0.0)
                    nc.vector.memset(ht[:, :, WP - 1:WP], 0.0)
                    for r0 in range(0, R, 8):
                        pss = [pspool.tile([128, 2, 256], F32,
                                           tag=f"ps{p}", name=f"ps{p}")
                               for p in range(4)]
                        for dx in range(3):
                            for p in range(4):
                                r = r0 + 2 * p
                                nc.tensor.matmul(
                                    pss[p][:, :, :],
                                    s0t[32 * p:32 * p + 12, dx, :],
                                    xt[32 * p:32 * p + 12, r:r + 2,
                                       dx:dx + 256],
                                    start=(dx == 0), stop=(dx == 2),
                                    tile_position=(32 * p, 0))
                        for p in range(4):
                            r = r0 + 2 * p
                            nc.scalar.activation(
                                ht[:, r:r + 2, 1:W + 1], pss[p][:, :, :], AF.Sign)
                    nc.scalar.dma_start(
                        out=h0_d[b, :, s * R + 1:s * R + R + 1, :],
                        in_=ht[:, :, :])

                # ---- stage B: conv1 (32 -> 32ch), K=128, M=128 ----
                for s in range(H // R if 'B' in stages else 0):
                    hin = apool.tile([128, R + 2, WP], BF16, tag="lin")
                    nc.sync.dma_start(
                        out=hin[:, :, :], in_=h0_d[b, :, s * R:s * R + R + 2, :])
                    ht = opool.tile([128, R, WP], BF16, tag="a_out")
                    nc.vector.memset(ht[:, :, 0:1], 0.0)
                    nc.vector.memset(ht[:, :, WP - 1:WP], 0.0)

                    def evict_b(pss, r0, ht=ht):
                        for p in range(4):
                            r = r0 + 2 * p
                            nc.scalar.activation(
                                ht[:, r:r + 2, 1:W + 1], pss[p][:, :, :], AF.Sign)

                    _conv_strip(nc, pspool,
                                [s1t[:, t, :] for t in range(9)], hin, evict_b, R)
                    nc.scalar.dma_start(
                        out=h1_d[b, :, s * R + 1:s * R + R + 1, :],
                        in_=ht[:, :, :])

                # ---- stage C: conv2 (32 -> 1ch), K=128, M=4, 4x col-tiling ----
                # 4 psum row-pairs go to col groups 0/32/64/96 of the SAME
                # psum tile; the 4 matmuls per tap run concurrently on
                # distinct 32-col strips of the PE array.
                for s in range(H // RC if 'C' in stages else 0):
                    hin = apool.tile([128, RC + 2, WP], BF16, tag="lin")
                    nc.sync.dma_start(
                        out=hin[:, :, :], in_=h1_d[b, :, s * RC:s * RC + RC + 2, :])
                    ot = cpool2.tile([B, RC, W], F32, tag="c_out")
                    for r0 in range(0, RC, 8):
                        ps = pspool.tile([128, 2, 256], F32, tag="ps0", name="psc")
                        for tap in range(9):
                            dy, dx = tap // 3, tap % 3
                            for p in range(4):
                                r = r0 + 2 * p
                                nc.tensor.matmul(
                                    ps[32 * p:32 * p + B, :, :],
                                    s2t[:, tap, :],
                                    hin[:, r + dy:r + dy + 2, dx:dx + 256],
                                    start=(tap == 0), stop=(tap == 8),
                                    tile_position=(0, 32 * p))
                        for p in range(4):
                            r = r0 + 2 * p
                            nc.vector.tensor_copy(
                                ot[:, r:r + 2, :], ps[32 * p:32 * p + B, :, :])
                    for g in range(B):
                        nc.scalar.dma_start(
                            out=out_d[b * B + g, s * RC:s * RC + RC, :],
                            in_=ot[g:g + 1, :, :])
    nc.compile()
    return nc


def _host_weights(w0, w1, w2):
    """Build bf16 block-diag stationary matrices. tap index = dy*3+dx."""
    sg = lambda w: np.sign(w).astype(ml_dtypes.bfloat16)
    w0s, w1s, w2s = sg(w0), sg(w1), sg(w2)   # [32,1,3,3],[32,32,3,3],[1,32,3,3]
    s0 = np.zeros((12, 3, 128), ml_dtypes.bfloat16)
    s1 = np.zeros((128, 9, 128), ml_dtypes.bfloat16)
    s2 = np.zeros((128, 9, B), ml_dtypes.bfloat16)
    for g in range(B):
        for dy in range(3):
            for dx in range(3):
                t = dy * 3 + dx
                # lhsT[k, m]: out[m] += sum_k lhsT[k,m]*rhs[k]
                # s0 [dy*4+g, dx, g*32+co]
                s0[dy * B + g, dx, g * 32:(g + 1) * 32] = w0s[:, 0, dy, dx]
                s1[g * 32:(g + 1) * 32, t, g * 32:(g + 1) * 32] = \
                    w1s[:, :, dy, dx].T  # [ci, co]
                s2[g * 32:(g + 1) * 32, t, g] = w2s[0, :, dy, dx]
    return s0, s1, s2


_NC_CACHE = {}


def _make_runner(stages=('0', 'A', 'B', 'C')):
    """Build + compile the SPMD program ONCE and return a cached callable.

    run_bass_kernel_spmd re-creates a fresh jax.jit(shard_map(...)) on every
    call, which re-runs tracing, HLO lowering and the whole
    bir_verify_and_optimise pipeline (~1.3 s/call) even when the program is
    unchanged. Caching the jitted function makes warm calls hit jax's pjit
    fast path, so they only pay input upload + device execution + download.
    """
    key = ("runner", stages)
    if key in _NC_CACHE:
        return _NC_CACHE[key]

    import jax
    from jax.sharding import Mesh, PartitionSpec
    from jax.experimental.shard_map import shard_map
    from concourse import bass2jax

    nc = _build_program(stages=stages)
    bass2jax.install_neuronx_cc_hook()

    in_names, out_names, out_avals = [], [], []
    for alloc in nc.m.functions[0].allocations:
        if not isinstance(alloc, mybir.MemoryLocationSet):
            continue
        name = alloc.memorylocations[0].name
        if alloc.kind == "ExternalInput":
            if nc.partition_id_tensor is None or name != nc.partition_id_tensor.name:
                in_names.append(name)
        elif alloc.kind == "ExternalOutput":
            out_names.append(name)
            out_avals.append(jax.core.ShapedArray(
                tuple(alloc.tensor_shape), mybir.dt.np(alloc.dtype)))
    n_params = len(in_names)
    n_outs = len(out_names)
    all_in_names = tuple(in_names) + tuple(out_names)
    if nc.partition_id_tensor is not None:
        all_in_names = all_in_names + (nc.partition_id_tensor.name,)
    donate = tuple(range(n_params, n_params + n_outs))

    def _body(*args):
        operands = list(args)
        if nc.partition_id_tensor is not None:
            operands.append(bass2jax.partition_id_tensor())
        outs = bass2jax._bass_exec_p.bind(
            *operands,
            out_avals=tuple(out_avals),
            in_names=all_in_names,
            out_names=tuple(out_names),
            lowering_input_output_aliases=(),
            sim_require_finite=True,
            sim_require_nnan=True,
            nc=nc,
        )
        return tuple(outs)

    devices = jax.devices()[:N_CORES]
    mesh = Mesh(np.asarray(devices), ("core",))
    sharded = jax.jit(
        shard_map(_body, mesh=mesh,
                  in_specs=(PartitionSpec("core"),) * (n_params + n_outs),
                  out_specs=(PartitionSpec("core"),) * n_outs,
                  check_rep=False),
        donate_argnums=donate, keep_unused=True)

    out_shapes = [tuple(a.shape) for a in out_avals]
    out_dtypes = [a.dtype for a in out_avals]

    def run(in_map_global):
        """in_map_global: dict name -> globally concatenated (8x) array."""
        args = [in_map_global[name] for name in in_names]
        zeros = [np.zeros((N_CORES * s[0],) + s[1:], d)
                 for s, d in zip(out_shapes, out_dtypes)]
        out_arrs = sharded(*args, *zeros)
        return {name: np.asarray(out_arrs[i]) for i, name in enumerate(out_names)}

    _NC_CACHE[key] = (nc, run)
    return _NC_CACHE[key]


def _global_inputs(x, w0, w1, w2):
    """Full inputs -> globally concatenated per-core input map (8 cores on
    axis 0, matching shard_map's P('core') sharding)."""
    s0, s1, s2 = _host_weights(np.asarray(w0), np.asarray(w1), np.asarray(w2))
    x = np.ascontiguousarray(np.asarray(x, np.float32).reshape(64, H, W))
    return {
        "x": x,                              # core i gets images 8i..8i+7
        "s0": np.tile(s0, (N_CORES, 1, 1)),
        "s1": np.tile(s1, (N_CORES, 1, 1)),
        "s2": np.tile(s2, (N_CORES, 1, 1)),
    }


def kernel(x, w0, w1, w2):
    try:
        _, run = _make_runner()
        out = run(_global_inputs(x, w0, w1, w2))["out"]
        return np.asarray(out, np.float32).reshape(64, 1, H, W)
    except Exception:
        if _NC_CACHE.get("fast_failed"):
            raise
        _NC_CACHE["fast_failed"] = True
        return _kernel_slowpath(x, w0, w1, w2)


def _kernel_slowpath(x, w0, w1, w2):
    if "nc" not in _NC_CACHE:
        _NC_CACHE["nc"] = _build_program()
    nc = _NC_CACHE["nc"]
    s0, s1, s2 = _host_weights(np.asarray(w0), np.asarray(w1), np.asarray(w2))
    x = np.asarray(x, np.float32).reshape(64, H, W)
    in_maps = [
        {"x": np.ascontiguousarray(x[i * IMG_PER_CORE:(i + 1) * IMG_PER_CORE]),
         "s0": s0, "s1": s1, "s2": s2}
        for i in range(N_CORES)
    ]
    res = run_bass_kernel_spmd(nc, in_maps, list(range(N_CORES)))
    out = np.stack([np.asarray(res.results[i]["out"], np.float32)
                    for i in range(N_CORES)])
    return out.reshape(64, 1, H, W)

